# revision 15
# baseline (speedup 1.0000x reference)
"""Distributed Trainium2 kernel for nn_Contrast_loss (row-parallel InfoNCE).

Math (reference):
  h1 = proj(pri), h2 = proj(aux)   with proj(z) = elu(z@W1.T+b1)@W2.T+b2
  n1 = normalize(h1), n2 = normalize(h2)
  l1_i = log(den1_i) - 2*d12_i,  den1_i = sum_j e^{2 S11_ij} + sum_j e^{2 S12_ij} - e^{2 S11_ii}
  l2_i = log(den2_i) - 2*d12_i,  den2_i = sum_j e^{2 S22_ij} + sum_j e^{2 S12_ji} - e^{2 S22_ii}
  loss = mean((l1+l2)/2)
  (S11_ii = S22_ii = 1 since rows are unit-normalized; d12_i = n1_i . n2_i)

Sharding: rows split across 8 cores (1024 rows each). Each core projects +
normalizes its row block (transposed layout [D, rows]) in bf16 matmuls, then
stores 16*n as fp8e4 tiles laid out [128, KC, R] (contraction chunk in dim1).
The two normalized matrices are AllGathered separately (4MB each) so the
second gather overlaps the S11 phase. Similarity row-blocks use fp8 DoubleRow
matmuls (2 contraction chunks per instruction, 2x PE throughput); exp(2x) and
row sums are fused on the scalar engine (activation accum_out, scale=2/256
since fp8 values carry a 16x scale -> psum holds 256*S). S12 column partials
accumulate on the vector engine and are partition-reduced with ones-matmuls.
Per-core partial sums are assembled into the scalar loss on CPU (O(N) work).

Normalization uses 1/sqrt(x) = exp(-0.5*ln(x) + ln(16)) so every activation
in the kernel (elu's exp/relu, ln, exp, identity) lives in the single
natural_log_exp_and_others table set -- no table reloads.
"""

import os
import numpy as np
import ml_dtypes

import concourse.bass as bass
import concourse.tile as tile
from concourse import mybir, bacc
from concourse.bass_utils import run_bass_kernel_spmd

NCORES = 8
N = 8192
D = 512
R = N // NCORES          # rows per core = 1024
KC = D // 128            # contraction chunks = 4
MT = R // 128            # row tiles per core = 8
BB = 4                   # column super-blocks (each = 2048 cols = 2 source cores)
F32 = mybir.dt.float32
BF16 = mybir.dt.bfloat16
F8 = mybir.dt.float8e4

FP8_SCALE = 16.0         # normalized values stored as 16*n in fp8e4
LN_SCALE = float(np.log(FP8_SCALE))

_CACHE = {}


def _build():
    STAGE = int(os.environ.get("BASS_STAGE", "4"))
    nc = bacc.Bacc("TRN2", target_bir_lowering=False, debug=False,
                   num_devices=NCORES)

    z1t = nc.dram_tensor("z1t", [KC, 128, R], BF16, kind="ExternalInput")
    z2t = nc.dram_tensor("z2t", [KC, 128, R], BF16, kind="ExternalInput")
    w1t = nc.dram_tensor("w1t", [KC, 128, D], BF16, kind="ExternalInput")
    w2t = nc.dram_tensor("w2t", [KC, 128, D], BF16, kind="ExternalInput")
    b1c = nc.dram_tensor("b1c", [128, KC], F32, kind="ExternalInput")
    b2c = nc.dram_tensor("b2c", [128, KC], F32, kind="ExternalInput")

    rs_out = nc.dram_tensor("rs", [128, 3 * MT * BB], F32, kind="ExternalOutput")
    cs_out = nc.dram_tensor("colsum", [16, 512], F32, kind="ExternalOutput")
    d12_out = nc.dram_tensor("d12", [2, 512], F32, kind="ExternalOutput")

    # per-e gather buffers (separate tensors so the two collectives carry no
    # false dependencies on each other)
    n_all = [nc.dram_tensor(f"n_all{e}", [NCORES, 128, KC, R], F8,
                            addr_space="Shared") for e in range(2)]
    warm_out = nc.dram_tensor("warm_out", [NCORES, 64], F8, addr_space="Shared")

    EXP = mybir.ActivationFunctionType.Exp
    RELU = mybir.ActivationFunctionType.Relu
    LNF = mybir.ActivationFunctionType.Ln
    IDENT = mybir.ActivationFunctionType.Identity
    DR = mybir.MatmulPerfMode.DoubleRow

    with tile.TileContext(nc) as tc:
        with tc.tile_pool(name="keep", bufs=1) as kp, \
             tc.tile_pool(name="dr", bufs=1, space="DRAM") as dr:

            # ---- persistent tiles ----
            b1s = kp.tile([128, KC], F32, name="b1s", tag="b1s")
            b2s = kp.tile([128, KC], F32, name="b2s", tag="b2s")
            nc.sync.dma_start(out=b1s, in_=b1c[:, :])
            nc.sync.dma_start(out=b2s, in_=b2c[:, :])
            ones_k = kp.tile([128, 1], F32, name="ones_k", tag="ones_k")
            nc.vector.memset(ones_k, 1.0)
            rs = kp.tile([128, 3 * MT * BB], F32, name="rs", tag="rs")
            nc.vector.memset(rs, 0.0)
            # fp8 normalized tiles (x16), contraction chunk on dim1
            nt8 = [kp.tile([128, KC, R], F8, name=f"nt8_{e}", tag=f"nt8_{e}")
                   for e in range(2)]
            mp = kp.tile([128, R], F32, name="mp", tag="mp")
            n_loc = [dr.tile([128, KC, R], F8, name=f"n_loc{e}", tag=f"n_loc{e}")
                     for e in range(2)]

            # tiny warm-up gather: pays the one-time RDH ring setup (~11us)
            # during the projection so the real gathers trigger fast
            warm_in = dr.tile([1, 64], F8, name="warm_in", tag="warm_in")
            if STAGE >= 2:
                nc.gpsimd.collective_compute(
                    "AllGather", mybir.AluOpType.bypass,
                    replica_groups=[list(range(NCORES))],
                    ins=[warm_in[:].opt()],
                    outs=[warm_out[:].opt()])

            # ---- projection + normalize (scoped pool) ----
            with tc.tile_pool(name="proj", bufs=1) as pj, \
                 tc.tile_pool(name="psp", bufs=1, space="PSUM") as psp:
                # batched input DMAs, ordered so e0's operands land first
                w1b = pj.tile([128, KC, D], BF16, name="w1b", tag="w1b")
                w2b = pj.tile([128, KC, D], BF16, name="w2b", tag="w2b")
                ztb = [pj.tile([128, KC, R], BF16, name=f"ztb{e}",
                               tag=f"ztb{e}") for e in range(2)]
                nc.sync.dma_start(out=w1b, in_=w1t[:])
                nc.sync.dma_start(out=ztb[0], in_=z1t[:])
                nc.sync.dma_start(out=ztb[1], in_=z2t[:])
                nc.sync.dma_start(out=w2b, in_=w2t[:])
                w1 = [w1b[:, k, :] for k in range(KC)]
                w2 = [w2b[:, k, :] for k in range(KC)]
                zt = [[ztb[e][:, k, :] for k in range(KC)] for e in range(2)]
                # broadcast vector carries the fp8 16x scale: bc = 16/|h|
                ones_b = pj.tile([1, 128], F32, name="ones_b", tag="ones_b")
                nc.vector.memset(ones_b, FP8_SCALE)

                # layer 1 + elu (elu = min(exp(x)-1, relu(x)))
                et = [[pj.tile([128, R], BF16, name=f"et{e}_{k}",
                               tag=f"et{e}_{k}") for k in range(KC)]
                      for e in range(2)]
                for e in range(2):
                    for oc in range(KC):
                        pa = psp.tile([128, R], F32, name="pa", tag="pa", bufs=2)
                        for h in range(R // 512):
                            for k in range(KC):
                                nc.tensor.matmul(
                                    pa[:, h * 512:(h + 1) * 512],
                                    w1[k][:, oc * 128:(oc + 1) * 128],
                                    zt[e][k][:, h * 512:(h + 1) * 512],
                                    start=(k == 0), stop=(k == KC - 1))
                        t1 = pj.tile([128, R], F32, name="t1", tag="t1", bufs=2)
                        t2 = pj.tile([128, R], F32, name="t2", tag="t2", bufs=2)
                        nc.scalar.activation(t1, pa, EXP, bias=b1s[:, oc:oc + 1])
                        nc.scalar.activation(t2, pa, RELU, bias=b1s[:, oc:oc + 1])
                        nc.vector.scalar_tensor_tensor(
                            et[e][oc], t1, 1.0, t2,
                            mybir.AluOpType.subtract, mybir.AluOpType.min)

                # layer 2 + bias; e0's squared norms fused into its loop
                ht = [[pj.tile([128, R], F32, name=f"ht{e}_{k}",
                               tag=f"ht{e}_{k}") for k in range(KC)]
                      for e in range(2)]
                nsq = [pj.tile([128, R], F32, name=f"nsq{e}", tag=f"nsq{e}")
                       for e in range(2)]
                for e in range(2):
                    for pc in range(KC):
                        ph = psp.tile([128, R], F32, name="pa", tag="pa", bufs=2)
                        for h in range(R // 512):
                            for k in range(KC):
                                nc.tensor.matmul(
                                    ph[:, h * 512:(h + 1) * 512],
                                    w2[k][:, pc * 128:(pc + 1) * 128],
                                    et[e][k][:, h * 512:(h + 1) * 512],
                                    start=(k == 0), stop=(k == KC - 1))
                        nc.scalar.activation(ht[e][pc], ph, IDENT,
                                             bias=b2s[:, pc:pc + 1])
                        if e == 0:
                            if pc == 0:
                                nc.vector.tensor_mul(nsq[0], ht[0][0], ht[0][0])
                            else:
                                sq = pj.tile([128, R], F32, name="t1",
                                             tag="t1", bufs=2)
                                nc.vector.tensor_mul(sq, ht[0][pc], ht[0][pc])
                                nc.vector.tensor_add(nsq[0], nsq[0], sq)

                def normalize_and_gather(e):
                    # 16/norm via exp(-0.5*ln(sum h^2)); broadcast; fp8 cast
                    nrm = psp.tile([1, R], F32, name="nrm", tag="nrm", bufs=1)
                    for h in range(R // 512):
                        nc.tensor.matmul(nrm[0:1, h * 512:(h + 1) * 512],
                                         ones_k,
                                         nsq[e][:, h * 512:(h + 1) * 512],
                                         start=True, stop=True)
                    snrm = pj.tile([1, R], F32, name="snrm", tag="snrm", bufs=2)
                    nc.vector.tensor_copy(snrm, nrm)
                    sr = pj.tile([1, R], F32, name="sr", tag="sr", bufs=2)
                    nc.scalar.activation(sr, snrm, LNF)
                    nc.scalar.activation(sr, sr, EXP, scale=-0.5)
                    bc = psp.tile([128, R], F32, name="bc", tag="bc", bufs=1)
                    for h in range(R // 512):
                        nc.tensor.matmul(bc[:, h * 512:(h + 1) * 512],
                                         ones_b,
                                         sr[0:1, h * 512:(h + 1) * 512],
                                         start=True, stop=True)
                    for pc in range(KC):
                        nc.vector.tensor_mul(nt8[e][:, pc, :], ht[e][pc], bc)
                    # DMA from the gpsimd queue so the sync queue (res tile
                    # loads) never waits behind this trigger
                    nc.gpsimd.dma_start(out=n_loc[e][:], in_=nt8[e][:])
                    if STAGE >= 2:
                        nc.gpsimd.collective_compute(
                            "AllGather", mybir.AluOpType.bypass,
                            replica_groups=[list(range(NCORES))],
                            ins=[n_loc[e][:].opt()],
                            outs=[n_all[e][:].opt()])

                normalize_and_gather(0)
                # e1's squared norms only now (keeps them off gather0's path)
                for pc in range(KC):
                    if pc == 0:
                        nc.vector.tensor_mul(nsq[1], ht[1][0], ht[1][0])
                    else:
                        sq = pj.tile([128, R], F32, name="t1", tag="t1", bufs=2)
                        nc.vector.tensor_mul(sq, ht[1][pc], ht[1][pc])
                        nc.vector.tensor_add(nsq[1], nsq[1], sq)
                normalize_and_gather(1)

                # d12 row-dot products from the fp8 tiles (256x scale)
                m2 = pj.tile([128, R], F32, name="t2", tag="t2", bufs=2)
                nc.vector.tensor_mul(mp, nt8[0][:, 0, :], nt8[1][:, 0, :])
                for k in range(1, KC):
                    nc.vector.tensor_mul(m2, nt8[0][:, k, :], nt8[1][:, k, :])
                    nc.vector.tensor_add(mp, mp, m2)

            # ---- main similarity loops (scoped pool) ----
            with tc.tile_pool(name="main", bufs=1) as mn:
              with tc.tile_pool(name="psm", bufs=1, space="PSUM") as psm:
                acc = mn.tile([128, N], F32, name="acc", tag="acc")
                nc.vector.memset(acc, 0.0)

                # d12 partition-reduce through a pg-slot while the tensor
                # engine is otherwise waiting on gather0
                dpg = psm.tile([128, 2048], F32, name="pg", tag="pg", bufs=2)
                for h in range(2):
                    nc.tensor.matmul(dpg[0:1, h * 512:(h + 1) * 512], ones_k,
                                     mp[:, h * 512:(h + 1) * 512],
                                     start=True, stop=True)
                dstg = mn.tile([1, 1024], F32, name="dstg", tag="dstg")
                nc.vector.tensor_copy(dstg, dpg[0:1, 0:1024])
                nc.sync.dma_start(out=d12_out[:, :], in_=dstg)

                def mm_group(pg, own, res, m):
                    for t in range(4):
                        j, half = t // 2, t % 2
                        for kq in range(2):
                            nc.tensor.matmul(
                                pg[:, t * 512:(t + 1) * 512],
                                own[:, 2 * kq:2 * kq + 2, m * 128:(m + 1) * 128],
                                res[j][:, 2 * kq:2 * kq + 2,
                                       half * 512:(half + 1) * 512],
                                start=(kq == 0), stop=(kq == 1),
                                perf_mode=DR)

                ESC = 2.0 / (FP8_SCALE * FP8_SCALE)  # psum holds 256*S

                # phase B: S11 (rhs = gathered n1)
                for bb in (range(BB) if STAGE >= 2 else []):
                    res = [mn.tile([128, KC, R], F8, name=f"r{j}",
                                   tag=f"r{j}", bufs=2) for j in range(2)]
                    for j in range(2):
                        nc.sync.dma_start(out=res[j][:],
                                          in_=n_all[0][2 * bb + j])
                    for m in range(MT):
                        pg = psm.tile([128, 2048], F32, name="pg", tag="pg",
                                      bufs=2)
                        mm_group(pg, nt8[0], res, m)
                        col = 0 * (MT * BB) + m * BB + bb
                        nc.scalar.activation(pg, pg, EXP, scale=ESC,
                                             accum_out=rs[:, col:col + 1])

                # phase CD: S12 + S22 (rhs = gathered n2)
                for bb in (range(BB) if STAGE >= 3 else []):
                    res = [mn.tile([128, KC, R], F8, name=f"r{j}",
                                   tag=f"r{j}", bufs=2) for j in range(2)]
                    for j in range(2):
                        nc.sync.dma_start(out=res[j][:],
                                          in_=n_all[1][2 * bb + j])
                    for m in range(MT):
                        pg = psm.tile([128, 2048], F32, name="pg", tag="pg",
                                      bufs=2)
                        mm_group(pg, nt8[0], res, m)
                        col = 1 * (MT * BB) + m * BB + bb
                        # stage exp in SBUF so the PSUM buffer frees at
                        # ACT-time and the column-sum add runs off-path
                        ex = mn.tile([128, 2048], F32, name="ex", tag="ex",
                                     bufs=2)
                        nc.scalar.activation(ex, pg, EXP, scale=ESC,
                                             accum_out=rs[:, col:col + 1])
                        nc.vector.tensor_add(acc[:, bb * 2048:(bb + 1) * 2048],
                                             acc[:, bb * 2048:(bb + 1) * 2048],
                                             ex)

                        pg2 = psm.tile([128, 2048], F32, name="pg", tag="pg",
                                       bufs=2)
                        mm_group(pg2, nt8[1], res, m)
                        col = 2 * (MT * BB) + m * BB + bb
                        nc.scalar.activation(pg2, pg2, EXP, scale=ESC,
                                             accum_out=rs[:, col:col + 1])

                    # this bb's column block of acc is complete: partition-
                    # reduce it now through a pg-slot (keeps the tail empty)
                    cpg = psm.tile([128, 2048], F32, name="pg", tag="pg",
                                   bufs=2)
                    for j in range(4):
                        nc.tensor.matmul(
                            cpg[0:1, j * 512:(j + 1) * 512], ones_k,
                            acc[:, bb * 2048 + j * 512:
                                bb * 2048 + (j + 1) * 512],
                            start=True, stop=True)
                    cstg = mn.tile([1, 2048], F32, name="cstg", tag="cstg",
                                   bufs=2)
                    nc.vector.tensor_copy(cstg, cpg[0:1, :])
                    nc.sync.dma_start(out=cs_out[bb * 4:(bb + 1) * 4, :],
                                      in_=cstg)
            nc.sync.dma_start(out=rs_out[:, :], in_=rs)

    nc.compile()
    return nc


def _get_nc():
    if "nc" not in _CACHE:
        _CACHE["nc"] = _build()
    return _CACHE["nc"]


def make_in_maps(pri, aux, W1, b1, W2, b2):
    pri = np.asarray(pri, dtype=np.float32)
    aux = np.asarray(aux, dtype=np.float32)
    w1t = np.ascontiguousarray(
        np.asarray(W1, dtype=np.float32).T).astype(
            ml_dtypes.bfloat16).reshape(KC, 128, D)
    w2t = np.ascontiguousarray(
        np.asarray(W2, dtype=np.float32).T).astype(
            ml_dtypes.bfloat16).reshape(KC, 128, D)
    b1 = np.asarray(b1, dtype=np.float32)
    b2 = np.asarray(b2, dtype=np.float32)
    b1c = np.ascontiguousarray(b1.reshape(KC, 128).T)
    b2c = np.ascontiguousarray(b2.reshape(KC, 128).T)
    priT = np.ascontiguousarray(pri.T).astype(ml_dtypes.bfloat16)
    auxT = np.ascontiguousarray(aux.T).astype(ml_dtypes.bfloat16)

    in_maps = []
    for c in range(NCORES):
        sl = slice(c * R, (c + 1) * R)
        in_maps.append({
            "z1t": np.ascontiguousarray(priT[:, sl]).reshape(KC, 128, R),
            "z2t": np.ascontiguousarray(auxT[:, sl]).reshape(KC, 128, R),
            "w1t": w1t, "w2t": w2t, "b1c": b1c, "b2c": b2c,
        })
    return in_maps


def assemble(results):
    """CPU assembly of the scalar loss from per-core partials"""
    E2 = np.exp(np.float64(2.0))
    SC2 = np.float64(FP8_SCALE * FP8_SCALE)
    colsum_full = np.zeros(N, dtype=np.float64)
    for c in range(NCORES):
        colsum_full += results[c]["colsum"].reshape(N).astype(np.float64)

    total = np.float64(0.0)
    for c in range(NCORES):
        rs = results[c]["rs"].astype(np.float64)      # [128, 96]
        r = rs.reshape(128, 3, MT, BB).sum(-1)        # [128, 3, MT]
        # row i_local = m*128 + p  ->  transpose to [MT, 128] then flatten
        rs11 = r[:, 0, :].T.reshape(R)
        rs12 = r[:, 1, :].T.reshape(R)
        rs22 = r[:, 2, :].T.reshape(R)
        d12 = results[c]["d12"].astype(np.float64).reshape(R) / SC2
        den1 = rs11 + rs12 - E2
        den2 = rs22 + colsum_full[c * R:(c + 1) * R] - E2
        li = 0.5 * (np.log(den1) + np.log(den2)) - 2.0 * d12
        total += li.sum()

    return np.float32(total / N)


def kernel(pri_embedding, aux_embedding, W1, b1, W2, b2):
    in_maps = make_in_maps(pri_embedding, aux_embedding, W1, b1, W2, b2)
    nc = _get_nc()
    res = run_bass_kernel_spmd(nc, in_maps, list(range(NCORES))).results
    return assemble(res)


# revision 17
# speedup vs baseline: 1.0495x; 1.0495x over previous
"""Distributed Trainium2 kernel for nn_Contrast_loss (row-parallel InfoNCE).

Math (reference):
  h1 = proj(pri), h2 = proj(aux)   with proj(z) = elu(z@W1.T+b1)@W2.T+b2
  n1 = normalize(h1), n2 = normalize(h2)
  l1_i = log(den1_i) - 2*d12_i,  den1_i = sum_j e^{2 S11_ij} + sum_j e^{2 S12_ij} - e^{2 S11_ii}
  l2_i = log(den2_i) - 2*d12_i,  den2_i = sum_j e^{2 S22_ij} + sum_j e^{2 S12_ji} - e^{2 S22_ii}
  loss = mean((l1+l2)/2)
  (S11_ii = S22_ii = 1 since rows are unit-normalized; d12_i = n1_i . n2_i)

Sharding: rows split across 8 cores (1024 rows each). Each core projects +
normalizes its row block (transposed layout [D, rows]) in bf16 matmuls, then
stores 16*n as fp8e4 tiles laid out [128, KC, R] (contraction chunk in dim1).
The two normalized matrices are AllGathered separately (4MB each) so the
second gather overlaps the S11 phase. Similarity row-blocks use fp8 DoubleRow
matmuls (2 contraction chunks per instruction, 2x PE throughput); exp(2x) and
row sums are fused on the scalar engine (activation accum_out, scale=2/256
since fp8 values carry a 16x scale -> psum holds 256*S). S12 column partials
accumulate on the vector engine and are partition-reduced with ones-matmuls.
Per-core partial sums are assembled into the scalar loss on CPU (O(N) work).

Normalization uses 1/sqrt(x) = exp(-0.5*ln(x) + ln(16)) so every activation
in the kernel (elu's exp/relu, ln, exp, identity) lives in the single
natural_log_exp_and_others table set -- no table reloads.
"""

import os
import numpy as np
import ml_dtypes

import concourse.bass as bass
import concourse.tile as tile
from concourse import mybir, bacc, bass_isa
from concourse.bass_utils import run_bass_kernel_spmd

NCORES = 8
N = 8192
D = 512
R = N // NCORES          # rows per core = 1024
KC = D // 128            # contraction chunks = 4
MT = R // 128            # row tiles per core = 8
BB = 4                   # column super-blocks (each = 2048 cols = 2 source cores)
F32 = mybir.dt.float32
BF16 = mybir.dt.bfloat16
F8 = mybir.dt.float8e4

FP8_SCALE = 16.0         # normalized values stored as 16*n in fp8e4
LN_SCALE = float(np.log(FP8_SCALE))

_CACHE = {}


def _build():
    STAGE = int(os.environ.get("BASS_STAGE", "4"))
    nc = bacc.Bacc("TRN2", target_bir_lowering=False, debug=False,
                   num_devices=NCORES)

    z1t = nc.dram_tensor("z1t", [KC, 128, R], BF16, kind="ExternalInput")
    z2t = nc.dram_tensor("z2t", [KC, 128, R], BF16, kind="ExternalInput")
    w1t = nc.dram_tensor("w1t", [KC, 128, D], BF16, kind="ExternalInput")
    w2t = nc.dram_tensor("w2t", [KC, 128, D], BF16, kind="ExternalInput")
    b1c = nc.dram_tensor("b1c", [128, KC], F32, kind="ExternalInput")
    b2c = nc.dram_tensor("b2c", [128, KC], F32, kind="ExternalInput")

    rs_out = nc.dram_tensor("rs", [128, 3 * MT * BB], F32, kind="ExternalOutput")
    cs_out = nc.dram_tensor("colsum", [16, 512], F32, kind="ExternalOutput")
    d12_out = nc.dram_tensor("d12", [2, 512], F32, kind="ExternalOutput")

    # per-e gather buffers (separate tensors so the two collectives carry no
    # false dependencies on each other)
    n_all = [nc.dram_tensor(f"n_all{e}", [NCORES, 128, KC, R], F8,
                            addr_space="Shared") for e in range(2)]
    warm_out = nc.dram_tensor("warm_out", [NCORES, 64], F8, addr_space="Shared")

    EXP = mybir.ActivationFunctionType.Exp
    RELU = mybir.ActivationFunctionType.Relu
    LNF = mybir.ActivationFunctionType.Ln
    IDENT = mybir.ActivationFunctionType.Identity
    DR = mybir.MatmulPerfMode.DoubleRow

    with tile.TileContext(nc) as tc:
        with tc.tile_pool(name="keep", bufs=1) as kp, \
             tc.tile_pool(name="dr", bufs=1, space="DRAM") as dr:

            # ---- persistent tiles ----
            b1s = kp.tile([128, KC], F32, name="b1s", tag="b1s")
            b2s = kp.tile([128, KC], F32, name="b2s", tag="b2s")
            nc.sync.dma_start(out=b1s, in_=b1c[:, :])
            nc.sync.dma_start(out=b2s, in_=b2c[:, :])
            ones_k = kp.tile([128, 1], F32, name="ones_k", tag="ones_k")
            nc.vector.memset(ones_k, 1.0)
            rs = kp.tile([128, 3 * MT * BB], F32, name="rs", tag="rs")
            nc.vector.memset(rs, 0.0)
            # fp8 normalized tiles (x16), contraction chunk on dim1
            nt8 = [kp.tile([128, KC, R], F8, name=f"nt8_{e}", tag=f"nt8_{e}")
                   for e in range(2)]
            mp = kp.tile([128, R], F32, name="mp", tag="mp")
            n_loc = [dr.tile([128, KC, R], F8, name=f"n_loc{e}", tag=f"n_loc{e}")
                     for e in range(2)]

            # tiny warm-up gather: pays the one-time RDH ring setup (~11us)
            # during the projection so the real gathers trigger fast
            warm_in = dr.tile([1, 64], F8, name="warm_in", tag="warm_in")
            if STAGE >= 2:
                nc.gpsimd.collective_compute(
                    "AllGather", mybir.AluOpType.bypass,
                    replica_groups=[list(range(NCORES))],
                    ins=[warm_in[:].opt()],
                    outs=[warm_out[:].opt()])

            # ---- projection + normalize (scoped pool) ----
            with tc.tile_pool(name="proj", bufs=1) as pj, \
                 tc.tile_pool(name="psp", bufs=1, space="PSUM") as psp:
                # batched input DMAs, ordered so e0's operands land first
                w1b = pj.tile([128, KC, D], BF16, name="w1b", tag="w1b")
                w2b = pj.tile([128, KC, D], BF16, name="w2b", tag="w2b")
                ztb = [pj.tile([128, KC, R], BF16, name=f"ztb{e}",
                               tag=f"ztb{e}") for e in range(2)]
                nc.sync.dma_start(out=w1b, in_=w1t[:])
                nc.sync.dma_start(out=ztb[0], in_=z1t[:])
                nc.sync.dma_start(out=ztb[1], in_=z2t[:])
                nc.sync.dma_start(out=w2b, in_=w2t[:])
                w1 = [w1b[:, k, :] for k in range(KC)]
                w2 = [w2b[:, k, :] for k in range(KC)]
                zt = [[ztb[e][:, k, :] for k in range(KC)] for e in range(2)]
                # broadcast vector carries the fp8 16x scale: bc = 16/|h|
                ones_b = pj.tile([1, 128], F32, name="ones_b", tag="ones_b")
                nc.vector.memset(ones_b, FP8_SCALE)

                # layer 1 + elu (elu = min(exp(x)-1, relu(x)))
                et = [[pj.tile([128, R], BF16, name=f"et{e}_{k}",
                               tag=f"et{e}_{k}") for k in range(KC)]
                      for e in range(2)]
                for e in range(2):
                    for oc in range(KC):
                        pa = psp.tile([128, R], F32, name="pa", tag="pa", bufs=2)
                        for h in range(R // 512):
                            for k in range(KC):
                                nc.tensor.matmul(
                                    pa[:, h * 512:(h + 1) * 512],
                                    w1[k][:, oc * 128:(oc + 1) * 128],
                                    zt[e][k][:, h * 512:(h + 1) * 512],
                                    start=(k == 0), stop=(k == KC - 1))
                        t1 = pj.tile([128, R], F32, name="t1", tag="t1", bufs=2)
                        t2 = pj.tile([128, R], F32, name="t2", tag="t2", bufs=2)
                        nc.scalar.activation(t1, pa, EXP, bias=b1s[:, oc:oc + 1])
                        nc.scalar.activation(t2, pa, RELU, bias=b1s[:, oc:oc + 1])
                        nc.vector.scalar_tensor_tensor(
                            et[e][oc], t1, 1.0, t2,
                            mybir.AluOpType.subtract, mybir.AluOpType.min)

                # layer 2 + bias; e0's squared norms fused into its loop
                ht = [[pj.tile([128, R], F32, name=f"ht{e}_{k}",
                               tag=f"ht{e}_{k}") for k in range(KC)]
                      for e in range(2)]
                nsq = [pj.tile([128, R], F32, name=f"nsq{e}", tag=f"nsq{e}")
                       for e in range(2)]
                for e in range(2):
                    for pc in range(KC):
                        ph = psp.tile([128, R], F32, name="pa", tag="pa", bufs=2)
                        for h in range(R // 512):
                            for k in range(KC):
                                nc.tensor.matmul(
                                    ph[:, h * 512:(h + 1) * 512],
                                    w2[k][:, pc * 128:(pc + 1) * 128],
                                    et[e][k][:, h * 512:(h + 1) * 512],
                                    start=(k == 0), stop=(k == KC - 1))
                        nc.scalar.activation(ht[e][pc], ph, IDENT,
                                             bias=b2s[:, pc:pc + 1])
                        if e == 0:
                            if pc == 0:
                                nc.vector.tensor_mul(nsq[0], ht[0][0], ht[0][0])
                            else:
                                sq = pj.tile([128, R], F32, name="t1",
                                             tag="t1", bufs=2)
                                nc.vector.tensor_mul(sq, ht[0][pc], ht[0][pc])
                                nc.vector.tensor_add(nsq[0], nsq[0], sq)

                def normalize_and_gather(e):
                    # 16/norm via exp(-0.5*ln(sum h^2)); broadcast; fp8 cast
                    nrm = psp.tile([1, R], F32, name="nrm", tag="nrm", bufs=1)
                    for h in range(R // 512):
                        nc.tensor.matmul(nrm[0:1, h * 512:(h + 1) * 512],
                                         ones_k,
                                         nsq[e][:, h * 512:(h + 1) * 512],
                                         start=True, stop=True)
                    snrm = pj.tile([1, R], F32, name="snrm", tag="snrm", bufs=2)
                    nc.vector.tensor_copy(snrm, nrm)
                    sr = pj.tile([1, R], F32, name="sr", tag="sr", bufs=2)
                    nc.scalar.activation(sr, snrm, LNF)
                    nc.scalar.activation(sr, sr, EXP, scale=-0.5)
                    bc = psp.tile([128, R], F32, name="bc", tag="bc", bufs=1)
                    for h in range(R // 512):
                        nc.tensor.matmul(bc[:, h * 512:(h + 1) * 512],
                                         ones_b,
                                         sr[0:1, h * 512:(h + 1) * 512],
                                         start=True, stop=True)
                    for pc in range(KC):
                        nc.vector.tensor_mul(nt8[e][:, pc, :], ht[e][pc], bc)
                    # DMA from the gpsimd queue so the sync queue (res tile
                    # loads) never waits behind this trigger
                    nc.gpsimd.dma_start(out=n_loc[e][:], in_=nt8[e][:])
                    if STAGE >= 2:
                        nc.gpsimd.collective_compute(
                            "AllGather", mybir.AluOpType.bypass,
                            replica_groups=[list(range(NCORES))],
                            ins=[n_loc[e][:].opt()],
                            outs=[n_all[e][:].opt()])

                normalize_and_gather(0)
                # e1's squared norms only now (keeps them off gather0's path)
                for pc in range(KC):
                    if pc == 0:
                        nc.vector.tensor_mul(nsq[1], ht[1][0], ht[1][0])
                    else:
                        sq = pj.tile([128, R], F32, name="t1", tag="t1", bufs=2)
                        nc.vector.tensor_mul(sq, ht[1][pc], ht[1][pc])
                        nc.vector.tensor_add(nsq[1], nsq[1], sq)
                normalize_and_gather(1)

                # d12 row-dot products from the fp8 tiles (256x scale)
                m2 = pj.tile([128, R], F32, name="t2", tag="t2", bufs=2)
                nc.vector.tensor_mul(mp, nt8[0][:, 0, :], nt8[1][:, 0, :])
                for k in range(1, KC):
                    nc.vector.tensor_mul(m2, nt8[0][:, k, :], nt8[1][:, k, :])
                    nc.vector.tensor_add(mp, mp, m2)

            # ---- main similarity loops (scoped pool) ----
            with tc.tile_pool(name="main", bufs=1) as mn:
              with tc.tile_pool(name="psm", bufs=1, space="PSUM") as psm:
                acc = mn.tile([128, N], F32, name="acc", tag="acc")
                nc.vector.memset(acc, 0.0)

                # d12 partition-reduce through a pg-slot while the tensor
                # engine is otherwise waiting on gather0
                dpg = psm.tile([128, 2048], F32, name="pg", tag="pg", bufs=2)
                for h in range(2):
                    nc.tensor.matmul(dpg[0:1, h * 512:(h + 1) * 512], ones_k,
                                     mp[:, h * 512:(h + 1) * 512],
                                     start=True, stop=True)
                dstg = mn.tile([1, 1024], F32, name="dstg", tag="dstg")
                nc.vector.tensor_copy(dstg, dpg[0:1, 0:1024])
                nc.sync.dma_start(out=d12_out[:, :], in_=dstg)

                def mm_group(pg, own, res, m):
                    for t in range(4):
                        j, half = t // 2, t % 2
                        for kq in range(2):
                            nc.tensor.matmul(
                                pg[:, t * 512:(t + 1) * 512],
                                own[:, 2 * kq:2 * kq + 2, m * 128:(m + 1) * 128],
                                res[j][:, 2 * kq:2 * kq + 2,
                                       half * 512:(half + 1) * 512],
                                start=(kq == 0), stop=(kq == 1),
                                perf_mode=DR)

                ESC = 2.0 / (FP8_SCALE * FP8_SCALE)  # psum holds 256*S

                # phase B: S11 (rhs = gathered n1)
                for bb in (range(BB) if STAGE >= 2 else []):
                    res = [mn.tile([128, KC, R], F8, name=f"r{j}",
                                   tag=f"r{j}", bufs=2) for j in range(2)]
                    for j in range(2):
                        nc.sync.dma_start(out=res[j][:],
                                          in_=n_all[0][2 * bb + j])
                    for m in range(MT):
                        pg = psm.tile([128, 2048], F32, name="pg", tag="pg",
                                      bufs=2)
                        mm_group(pg, nt8[0], res, m)
                        col = 0 * (MT * BB) + m * BB + bb
                        nc.scalar.activation(pg, pg, EXP, scale=ESC,
                                             accum_out=rs[:, col:col + 1])

                # phase CD: S12 + S22 (rhs = gathered n2)
                for bb in (range(BB) if STAGE >= 3 else []):
                    res = [mn.tile([128, KC, R], F8, name=f"r{j}",
                                   tag=f"r{j}", bufs=2) for j in range(2)]
                    for j in range(2):
                        nc.sync.dma_start(out=res[j][:],
                                          in_=n_all[1][2 * bb + j])
                    for m in range(MT):
                        pg = psm.tile([128, 2048], F32, name="pg", tag="pg",
                                      bufs=2)
                        mm_group(pg, nt8[0], res, m)
                        col = 1 * (MT * BB) + m * BB + bb
                        # stage exp in SBUF so the PSUM buffer frees at
                        # ACT-time and the column-sum add runs off-path
                        ex = mn.tile([128, 2048], F32, name="ex", tag="ex",
                                     bufs=2)
                        nc.scalar.activation(ex, pg, EXP, scale=ESC,
                                             accum_out=rs[:, col:col + 1])
                        nc.vector.tensor_add(acc[:, bb * 2048:(bb + 1) * 2048],
                                             acc[:, bb * 2048:(bb + 1) * 2048],
                                             ex)

                        pg2 = psm.tile([128, 2048], F32, name="pg", tag="pg",
                                       bufs=2)
                        mm_group(pg2, nt8[1], res, m)
                        col = 2 * (MT * BB) + m * BB + bb
                        nc.scalar.activation(pg2, pg2, EXP, scale=ESC,
                                             accum_out=rs[:, col:col + 1])

                    # this bb's column block of acc is complete: partition-
                    # reduce it on the otherwise-idle gpsimd engine so the
                    # tensor/ACT pipeline never sees it
                    car = mn.tile([128, 2048], F32, name="car", tag="car",
                                  bufs=2)
                    nc.gpsimd.partition_all_reduce(
                        car, acc[:, bb * 2048:(bb + 1) * 2048],
                        channels=128, reduce_op=bass_isa.ReduceOp.add)
                    nc.sync.dma_start(out=cs_out[bb * 4:(bb + 1) * 4, :],
                                      in_=car[0:1, :])
            nc.sync.dma_start(out=rs_out[:, :], in_=rs)

    nc.compile()
    return nc


def _get_nc():
    if "nc" not in _CACHE:
        _CACHE["nc"] = _build()
    return _CACHE["nc"]


def make_in_maps(pri, aux, W1, b1, W2, b2):
    pri = np.asarray(pri, dtype=np.float32)
    aux = np.asarray(aux, dtype=np.float32)
    w1t = np.ascontiguousarray(
        np.asarray(W1, dtype=np.float32).T).astype(
            ml_dtypes.bfloat16).reshape(KC, 128, D)
    w2t = np.ascontiguousarray(
        np.asarray(W2, dtype=np.float32).T).astype(
            ml_dtypes.bfloat16).reshape(KC, 128, D)
    b1 = np.asarray(b1, dtype=np.float32)
    b2 = np.asarray(b2, dtype=np.float32)
    b1c = np.ascontiguousarray(b1.reshape(KC, 128).T)
    b2c = np.ascontiguousarray(b2.reshape(KC, 128).T)
    priT = np.ascontiguousarray(pri.T).astype(ml_dtypes.bfloat16)
    auxT = np.ascontiguousarray(aux.T).astype(ml_dtypes.bfloat16)

    in_maps = []
    for c in range(NCORES):
        sl = slice(c * R, (c + 1) * R)
        in_maps.append({
            "z1t": np.ascontiguousarray(priT[:, sl]).reshape(KC, 128, R),
            "z2t": np.ascontiguousarray(auxT[:, sl]).reshape(KC, 128, R),
            "w1t": w1t, "w2t": w2t, "b1c": b1c, "b2c": b2c,
        })
    return in_maps


def assemble(results):
    """CPU assembly of the scalar loss from per-core partials"""
    E2 = np.exp(np.float64(2.0))
    SC2 = np.float64(FP8_SCALE * FP8_SCALE)
    colsum_full = np.zeros(N, dtype=np.float64)
    for c in range(NCORES):
        colsum_full += results[c]["colsum"].reshape(N).astype(np.float64)

    total = np.float64(0.0)
    for c in range(NCORES):
        rs = results[c]["rs"].astype(np.float64)      # [128, 96]
        r = rs.reshape(128, 3, MT, BB).sum(-1)        # [128, 3, MT]
        # row i_local = m*128 + p  ->  transpose to [MT, 128] then flatten
        rs11 = r[:, 0, :].T.reshape(R)
        rs12 = r[:, 1, :].T.reshape(R)
        rs22 = r[:, 2, :].T.reshape(R)
        d12 = results[c]["d12"].astype(np.float64).reshape(R) / SC2
        den1 = rs11 + rs12 - E2
        den2 = rs22 + colsum_full[c * R:(c + 1) * R] - E2
        li = 0.5 * (np.log(den1) + np.log(den2)) - 2.0 * d12
        total += li.sum()

    return np.float32(total / N)


def kernel(pri_embedding, aux_embedding, W1, b1, W2, b2):
    in_maps = make_in_maps(pri_embedding, aux_embedding, W1, b1, W2, b2)
    nc = _get_nc()
    res = run_bass_kernel_spmd(nc, in_maps, list(range(NCORES))).results
    return assemble(res)


# revision 20
# speedup vs baseline: 1.0853x; 1.0341x over previous
"""Distributed Trainium2 kernel for nn_Contrast_loss (row-parallel InfoNCE).

Math (reference):
  h1 = proj(pri), h2 = proj(aux)   with proj(z) = elu(z@W1.T+b1)@W2.T+b2
  n1 = normalize(h1), n2 = normalize(h2)
  l1_i = log(den1_i) - 2*d12_i,  den1_i = sum_j e^{2 S11_ij} + sum_j e^{2 S12_ij} - e^{2 S11_ii}
  l2_i = log(den2_i) - 2*d12_i,  den2_i = sum_j e^{2 S22_ij} + sum_j e^{2 S12_ji} - e^{2 S22_ii}
  loss = mean((l1+l2)/2)
  (S11_ii = S22_ii = 1 since rows are unit-normalized; d12_i = n1_i . n2_i)

Sharding: rows split across 8 cores (1024 rows each). Each core projects +
normalizes its row block (transposed layout [D, rows]) in bf16 matmuls, then
stores 16*n as fp8e4 tiles laid out [128, KC, R] (contraction chunk in dim1).
The two normalized matrices are AllGathered separately (4MB each) so the
second gather overlaps the S11 phase. Similarity row-blocks use fp8 DoubleRow
matmuls (2 contraction chunks per instruction, 2x PE throughput); exp(2x) and
row sums are fused on the scalar engine (activation accum_out, scale=2/256
since fp8 values carry a 16x scale -> psum holds 256*S). S12 column partials
accumulate on the vector engine and are partition-reduced with ones-matmuls.
Per-core partial sums are assembled into the scalar loss on CPU (O(N) work).

Normalization uses 1/sqrt(x) = exp(-0.5*ln(x) + ln(16)) so every activation
in the kernel (elu's exp/relu, ln, exp, identity) lives in the single
natural_log_exp_and_others table set -- no table reloads.
"""

import os
import numpy as np
import ml_dtypes

import concourse.bass as bass
import concourse.tile as tile
from concourse import mybir, bacc, bass_isa
from concourse.bass_utils import run_bass_kernel_spmd

NCORES = 8
N = 8192
D = 512
R = N // NCORES          # rows per core = 1024
KC = D // 128            # contraction chunks = 4
MT = R // 128            # row tiles per core = 8
BB = 4                   # column super-blocks (each = 2048 cols = 2 source cores)
F32 = mybir.dt.float32
BF16 = mybir.dt.bfloat16
F8 = mybir.dt.float8e4

FP8_SCALE = 16.0         # normalized values stored as 16*n in fp8e4
LN_SCALE = float(np.log(FP8_SCALE))

_CACHE = {}


def _build():
    STAGE = int(os.environ.get("BASS_STAGE", "4"))
    nc = bacc.Bacc("TRN2", target_bir_lowering=False, debug=False,
                   num_devices=NCORES)

    z1t = nc.dram_tensor("z1t", [KC, 128, R], BF16, kind="ExternalInput")
    z2t = nc.dram_tensor("z2t", [KC, 128, R], BF16, kind="ExternalInput")
    w1t = nc.dram_tensor("w1t", [KC, 128, D], BF16, kind="ExternalInput")
    w2t = nc.dram_tensor("w2t", [KC, 128, D], BF16, kind="ExternalInput")
    b1c = nc.dram_tensor("b1c", [128, KC], F32, kind="ExternalInput")
    b2c = nc.dram_tensor("b2c", [128, KC], F32, kind="ExternalInput")

    rs_out = nc.dram_tensor("rs", [128, 3 * MT * BB], F32, kind="ExternalOutput")
    acc_out = nc.dram_tensor("accsum", [128, N], F32, kind="ExternalOutput")
    d12_out = nc.dram_tensor("d12", [2, 512], F32, kind="ExternalOutput")

    # per-e gather buffers (separate tensors so the two collectives carry no
    # false dependencies on each other)
    n_all = [nc.dram_tensor(f"n_all{e}", [NCORES, 128, KC, R], F8,
                            addr_space="Shared") for e in range(2)]
    warm_out = nc.dram_tensor("warm_out", [NCORES, 64], F8, addr_space="Shared")

    EXP = mybir.ActivationFunctionType.Exp
    RELU = mybir.ActivationFunctionType.Relu
    LNF = mybir.ActivationFunctionType.Ln
    IDENT = mybir.ActivationFunctionType.Identity
    DR = mybir.MatmulPerfMode.DoubleRow

    with tile.TileContext(nc) as tc:
        with tc.tile_pool(name="keep", bufs=1) as kp, \
             tc.tile_pool(name="dr", bufs=1, space="DRAM") as dr:

            # ---- persistent tiles ----
            b1s = kp.tile([128, KC], F32, name="b1s", tag="b1s")
            b2s = kp.tile([128, KC], F32, name="b2s", tag="b2s")
            nc.sync.dma_start(out=b1s, in_=b1c[:, :])
            nc.sync.dma_start(out=b2s, in_=b2c[:, :])
            ones_k = kp.tile([128, 1], F32, name="ones_k", tag="ones_k")
            nc.vector.memset(ones_k, 1.0)
            rs = kp.tile([128, 3 * MT * BB], F32, name="rs", tag="rs")
            nc.vector.memset(rs, 0.0)
            # fp8 normalized tiles (x16), contraction chunk on dim1
            nt8 = [kp.tile([128, KC, R], F8, name=f"nt8_{e}", tag=f"nt8_{e}")
                   for e in range(2)]
            mp = kp.tile([128, R], F32, name="mp", tag="mp")
            n_loc = [dr.tile([128, KC, R], F8, name=f"n_loc{e}", tag=f"n_loc{e}")
                     for e in range(2)]

            # tiny warm-up gather: pays the one-time RDH ring setup (~11us)
            # during the projection so the real gathers trigger fast
            warm_in = dr.tile([1, 64], F8, name="warm_in", tag="warm_in")
            if STAGE >= 2:
                nc.gpsimd.collective_compute(
                    "AllGather", mybir.AluOpType.bypass,
                    replica_groups=[list(range(NCORES))],
                    ins=[warm_in[:].opt()],
                    outs=[warm_out[:].opt()])

            # ---- projection + normalize (scoped pool) ----
            with tc.tile_pool(name="proj", bufs=1) as pj, \
                 tc.tile_pool(name="psp", bufs=1, space="PSUM") as psp:
                # batched input DMAs, ordered so e0's operands land first
                w1b = pj.tile([128, KC, D], BF16, name="w1b", tag="w1b")
                w2b = pj.tile([128, KC, D], BF16, name="w2b", tag="w2b")
                ztb = [pj.tile([128, KC, R], BF16, name=f"ztb{e}",
                               tag=f"ztb{e}") for e in range(2)]
                nc.sync.dma_start(out=w1b, in_=w1t[:])
                nc.sync.dma_start(out=ztb[0], in_=z1t[:])
                nc.sync.dma_start(out=ztb[1], in_=z2t[:])
                nc.sync.dma_start(out=w2b, in_=w2t[:])
                w1 = [w1b[:, k, :] for k in range(KC)]
                w2 = [w2b[:, k, :] for k in range(KC)]
                zt = [[ztb[e][:, k, :] for k in range(KC)] for e in range(2)]
                # broadcast vector carries the fp8 16x scale: bc = 16/|h|
                ones_b = pj.tile([1, 128], F32, name="ones_b", tag="ones_b")
                nc.vector.memset(ones_b, FP8_SCALE)

                # layer 1 + elu (elu = min(exp(x)-1, relu(x)))
                et = [[pj.tile([128, R], BF16, name=f"et{e}_{k}",
                               tag=f"et{e}_{k}") for k in range(KC)]
                      for e in range(2)]
                for e in range(2):
                    for oc in range(KC):
                        pa = psp.tile([128, R], F32, name="pa", tag="pa", bufs=2)
                        for h in range(R // 512):
                            for k in range(KC):
                                nc.tensor.matmul(
                                    pa[:, h * 512:(h + 1) * 512],
                                    w1[k][:, oc * 128:(oc + 1) * 128],
                                    zt[e][k][:, h * 512:(h + 1) * 512],
                                    start=(k == 0), stop=(k == KC - 1))
                        t1 = pj.tile([128, R], F32, name="t1", tag="t1", bufs=2)
                        t2 = pj.tile([128, R], F32, name="t2", tag="t2", bufs=2)
                        nc.scalar.activation(t1, pa, EXP, bias=b1s[:, oc:oc + 1])
                        nc.scalar.activation(t2, pa, RELU, bias=b1s[:, oc:oc + 1])
                        nc.vector.scalar_tensor_tensor(
                            et[e][oc], t1, 1.0, t2,
                            mybir.AluOpType.subtract, mybir.AluOpType.min)

                # layer 2 + bias; e0's squared norms fused into its loop
                ht = [[pj.tile([128, R], F32, name=f"ht{e}_{k}",
                               tag=f"ht{e}_{k}") for k in range(KC)]
                      for e in range(2)]
                nsq = [pj.tile([128, R], F32, name=f"nsq{e}", tag=f"nsq{e}")
                       for e in range(2)]
                for e in range(2):
                    for pc in range(KC):
                        ph = psp.tile([128, R], F32, name="pa", tag="pa", bufs=2)
                        for h in range(R // 512):
                            for k in range(KC):
                                nc.tensor.matmul(
                                    ph[:, h * 512:(h + 1) * 512],
                                    w2[k][:, pc * 128:(pc + 1) * 128],
                                    et[e][k][:, h * 512:(h + 1) * 512],
                                    start=(k == 0), stop=(k == KC - 1))
                        nc.scalar.activation(ht[e][pc], ph, IDENT,
                                             bias=b2s[:, pc:pc + 1])
                        if e == 0:
                            if pc == 0:
                                nc.vector.tensor_mul(nsq[0], ht[0][0], ht[0][0])
                            else:
                                sq = pj.tile([128, R], F32, name="t1",
                                             tag="t1", bufs=2)
                                nc.vector.tensor_mul(sq, ht[0][pc], ht[0][pc])
                                nc.vector.tensor_add(nsq[0], nsq[0], sq)

                def normalize_and_gather(e):
                    # 16/norm via exp(-0.5*ln(sum h^2)); broadcast; fp8 cast
                    nrm = psp.tile([1, R], F32, name="nrm", tag="nrm", bufs=1)
                    for h in range(R // 512):
                        nc.tensor.matmul(nrm[0:1, h * 512:(h + 1) * 512],
                                         ones_k,
                                         nsq[e][:, h * 512:(h + 1) * 512],
                                         start=True, stop=True)
                    snrm = pj.tile([1, R], F32, name="snrm", tag="snrm", bufs=2)
                    nc.vector.tensor_copy(snrm, nrm)
                    sr = pj.tile([1, R], F32, name="sr", tag="sr", bufs=2)
                    nc.scalar.activation(sr, snrm, LNF)
                    nc.scalar.activation(sr, sr, EXP, scale=-0.5)
                    bc = psp.tile([128, R], F32, name="bc", tag="bc", bufs=1)
                    for h in range(R // 512):
                        nc.tensor.matmul(bc[:, h * 512:(h + 1) * 512],
                                         ones_b,
                                         sr[0:1, h * 512:(h + 1) * 512],
                                         start=True, stop=True)
                    for pc in range(KC):
                        nc.vector.tensor_mul(nt8[e][:, pc, :], ht[e][pc], bc)
                    # DMA from the gpsimd queue so the sync queue (res tile
                    # loads) never waits behind this trigger
                    nc.gpsimd.dma_start(out=n_loc[e][:], in_=nt8[e][:])
                    if STAGE >= 2:
                        nc.gpsimd.collective_compute(
                            "AllGather", mybir.AluOpType.bypass,
                            replica_groups=[list(range(NCORES))],
                            ins=[n_loc[e][:].opt()],
                            outs=[n_all[e][:].opt()])

                normalize_and_gather(0)
                # e1's squared norms only now (keeps them off gather0's path)
                for pc in range(KC):
                    if pc == 0:
                        nc.vector.tensor_mul(nsq[1], ht[1][0], ht[1][0])
                    else:
                        sq = pj.tile([128, R], F32, name="t1", tag="t1", bufs=2)
                        nc.vector.tensor_mul(sq, ht[1][pc], ht[1][pc])
                        nc.vector.tensor_add(nsq[1], nsq[1], sq)
                normalize_and_gather(1)

                # d12 row-dot products from the fp8 tiles (256x scale)
                m2 = pj.tile([128, R], F32, name="t2", tag="t2", bufs=2)
                nc.vector.tensor_mul(mp, nt8[0][:, 0, :], nt8[1][:, 0, :])
                for k in range(1, KC):
                    nc.vector.tensor_mul(m2, nt8[0][:, k, :], nt8[1][:, k, :])
                    nc.vector.tensor_add(mp, mp, m2)

            # ---- main similarity loops (scoped pool) ----
            with tc.tile_pool(name="main", bufs=1) as mn:
              with tc.tile_pool(name="psm", bufs=1, space="PSUM") as psm:
                acc = mn.tile([128, N], F32, name="acc", tag="acc")
                nc.vector.memset(acc, 0.0)

                # d12 partition-reduce through a pg-slot while the tensor
                # engine is otherwise waiting on gather0
                dpg = psm.tile([128, 2048], F32, name="pg", tag="pg", bufs=2)
                for h in range(2):
                    nc.tensor.matmul(dpg[0:1, h * 512:(h + 1) * 512], ones_k,
                                     mp[:, h * 512:(h + 1) * 512],
                                     start=True, stop=True)
                dstg = mn.tile([1, 1024], F32, name="dstg", tag="dstg")
                nc.vector.tensor_copy(dstg, dpg[0:1, 0:1024])
                nc.sync.dma_start(out=d12_out[:, :], in_=dstg)

                def mm_group(pg, own, res, m):
                    for t in range(4):
                        j, half = t // 2, t % 2
                        for kq in range(2):
                            nc.tensor.matmul(
                                pg[:, t * 512:(t + 1) * 512],
                                own[:, 2 * kq:2 * kq + 2, m * 128:(m + 1) * 128],
                                res[j][:, 2 * kq:2 * kq + 2,
                                       half * 512:(half + 1) * 512],
                                start=(kq == 0), stop=(kq == 1),
                                perf_mode=DR)

                ESC = 2.0 / (FP8_SCALE * FP8_SCALE)  # psum holds 256*S

                # phase B: S11 (rhs = gathered n1)
                for bb in (range(BB) if STAGE >= 2 else []):
                    res = [mn.tile([128, KC, R], F8, name=f"r{j}",
                                   tag=f"r{j}", bufs=2) for j in range(2)]
                    for j in range(2):
                        nc.sync.dma_start(out=res[j][:],
                                          in_=n_all[0][2 * bb + j])
                    for m in range(MT):
                        pg = psm.tile([128, 2048], F32, name="pg", tag="pg",
                                      bufs=2)
                        mm_group(pg, nt8[0], res, m)
                        col = 0 * (MT * BB) + m * BB + bb
                        nc.scalar.activation(pg, pg, EXP, scale=ESC,
                                             accum_out=rs[:, col:col + 1])

                # phase CD: S12 + S22 (rhs = gathered n2)
                for bb in (range(BB) if STAGE >= 3 else []):
                    res = [mn.tile([128, KC, R], F8, name=f"r{j}",
                                   tag=f"r{j}", bufs=2) for j in range(2)]
                    for j in range(2):
                        nc.sync.dma_start(out=res[j][:],
                                          in_=n_all[1][2 * bb + j])
                    for m in range(MT):
                        pg = psm.tile([128, 2048], F32, name="pg", tag="pg",
                                      bufs=2)
                        mm_group(pg, nt8[0], res, m)
                        col = 1 * (MT * BB) + m * BB + bb
                        # stage exp in SBUF so the PSUM buffer frees at
                        # ACT-time and the column-sum add runs off-path
                        ex = mn.tile([128, 2048], F32, name="ex", tag="ex",
                                     bufs=2)
                        nc.scalar.activation(ex, pg, EXP, scale=ESC,
                                             accum_out=rs[:, col:col + 1])
                        nc.vector.tensor_add(acc[:, bb * 2048:(bb + 1) * 2048],
                                             acc[:, bb * 2048:(bb + 1) * 2048],
                                             ex)

                        pg2 = psm.tile([128, 2048], F32, name="pg", tag="pg",
                                       bufs=2)
                        mm_group(pg2, nt8[1], res, m)
                        col = 2 * (MT * BB) + m * BB + bb
                        nc.scalar.activation(pg2, pg2, EXP, scale=ESC,
                                             accum_out=rs[:, col:col + 1])

                    # this bb's column block of acc is complete: ship it to
                    # DRAM on idle DMA bandwidth; the 128-way partition
                    # reduction happens in the host assembly
                    nc.sync.dma_start(out=acc_out[:, bb * 2048:(bb + 1) * 2048],
                                      in_=acc[:, bb * 2048:(bb + 1) * 2048])
            nc.sync.dma_start(out=rs_out[:, :], in_=rs)

    nc.compile()
    return nc


def _get_nc():
    if "nc" not in _CACHE:
        _CACHE["nc"] = _build()
    return _CACHE["nc"]


def make_in_maps(pri, aux, W1, b1, W2, b2):
    pri = np.asarray(pri, dtype=np.float32)
    aux = np.asarray(aux, dtype=np.float32)
    w1t = np.ascontiguousarray(
        np.asarray(W1, dtype=np.float32).T).astype(
            ml_dtypes.bfloat16).reshape(KC, 128, D)
    w2t = np.ascontiguousarray(
        np.asarray(W2, dtype=np.float32).T).astype(
            ml_dtypes.bfloat16).reshape(KC, 128, D)
    b1 = np.asarray(b1, dtype=np.float32)
    b2 = np.asarray(b2, dtype=np.float32)
    b1c = np.ascontiguousarray(b1.reshape(KC, 128).T)
    b2c = np.ascontiguousarray(b2.reshape(KC, 128).T)
    priT = np.ascontiguousarray(pri.T).astype(ml_dtypes.bfloat16)
    auxT = np.ascontiguousarray(aux.T).astype(ml_dtypes.bfloat16)

    in_maps = []
    for c in range(NCORES):
        sl = slice(c * R, (c + 1) * R)
        in_maps.append({
            "z1t": np.ascontiguousarray(priT[:, sl]).reshape(KC, 128, R),
            "z2t": np.ascontiguousarray(auxT[:, sl]).reshape(KC, 128, R),
            "w1t": w1t, "w2t": w2t, "b1c": b1c, "b2c": b2c,
        })
    return in_maps


def assemble(results):
    """CPU assembly of the scalar loss from per-core partials"""
    E2 = np.exp(np.float64(2.0))
    SC2 = np.float64(FP8_SCALE * FP8_SCALE)
    colsum_full = np.zeros(N, dtype=np.float64)
    for c in range(NCORES):
        colsum_full += results[c]["accsum"].astype(np.float64).sum(axis=0)

    total = np.float64(0.0)
    for c in range(NCORES):
        rs = results[c]["rs"].astype(np.float64)      # [128, 96]
        r = rs.reshape(128, 3, MT, BB).sum(-1)        # [128, 3, MT]
        # row i_local = m*128 + p  ->  transpose to [MT, 128] then flatten
        rs11 = r[:, 0, :].T.reshape(R)
        rs12 = r[:, 1, :].T.reshape(R)
        rs22 = r[:, 2, :].T.reshape(R)
        d12 = results[c]["d12"].astype(np.float64).reshape(R) / SC2
        den1 = rs11 + rs12 - E2
        den2 = rs22 + colsum_full[c * R:(c + 1) * R] - E2
        li = 0.5 * (np.log(den1) + np.log(den2)) - 2.0 * d12
        total += li.sum()

    return np.float32(total / N)


def kernel(pri_embedding, aux_embedding, W1, b1, W2, b2):
    in_maps = make_in_maps(pri_embedding, aux_embedding, W1, b1, W2, b2)
    nc = _get_nc()
    res = run_bass_kernel_spmd(nc, in_maps, list(range(NCORES))).results
    return assemble(res)


# revision 22
# speedup vs baseline: 1.0922x; 1.0064x over previous
"""Distributed Trainium2 kernel for nn_Contrast_loss (row-parallel InfoNCE).

Math (reference):
  h1 = proj(pri), h2 = proj(aux)   with proj(z) = elu(z@W1.T+b1)@W2.T+b2
  n1 = normalize(h1), n2 = normalize(h2)
  l1_i = log(den1_i) - 2*d12_i,  den1_i = sum_j e^{2 S11_ij} + sum_j e^{2 S12_ij} - e^{2 S11_ii}
  l2_i = log(den2_i) - 2*d12_i,  den2_i = sum_j e^{2 S22_ij} + sum_j e^{2 S12_ji} - e^{2 S22_ii}
  loss = mean((l1+l2)/2)
  (S11_ii = S22_ii = 1 since rows are unit-normalized; d12_i = n1_i . n2_i)

Sharding: rows split across 8 cores (1024 rows each). Each core projects +
normalizes its row block (transposed layout [D, rows]) in bf16 matmuls, then
stores 16*n as fp8e4 tiles laid out [128, KC, R] (contraction chunk in dim1).
The two normalized matrices are AllGathered separately (4MB each) so the
second gather overlaps the S11 phase. Similarity row-blocks use fp8 DoubleRow
matmuls (2 contraction chunks per instruction, 2x PE throughput); exp(2x) and
row sums are fused on the scalar engine (activation accum_out, scale=2/256
since fp8 values carry a 16x scale -> psum holds 256*S). S12 column partials
accumulate on the vector engine and are partition-reduced with ones-matmuls.
Per-core partial sums are assembled into the scalar loss on CPU (O(N) work).

Normalization uses 1/sqrt(x) = exp(-0.5*ln(x) + ln(16)) so every activation
in the kernel (elu's exp/relu, ln, exp, identity) lives in the single
natural_log_exp_and_others table set -- no table reloads.
"""

import os
import numpy as np
import ml_dtypes

import concourse.bass as bass
import concourse.tile as tile
from concourse import mybir, bacc, bass_isa
from concourse.bass_utils import run_bass_kernel_spmd

NCORES = 8
N = 8192
D = 512
R = N // NCORES          # rows per core = 1024
KC = D // 128            # contraction chunks = 4
MT = R // 128            # row tiles per core = 8
BB = 4                   # column super-blocks (each = 2048 cols = 2 source cores)
F32 = mybir.dt.float32
BF16 = mybir.dt.bfloat16
F8 = mybir.dt.float8e4

FP8_SCALE = 16.0         # normalized values stored as 16*n in fp8e4
LN_SCALE = float(np.log(FP8_SCALE))

_CACHE = {}


def _build():
    STAGE = int(os.environ.get("BASS_STAGE", "4"))
    nc = bacc.Bacc("TRN2", target_bir_lowering=False, debug=False,
                   num_devices=NCORES)

    z1t = nc.dram_tensor("z1t", [KC, 128, R], BF16, kind="ExternalInput")
    z2t = nc.dram_tensor("z2t", [KC, 128, R], BF16, kind="ExternalInput")
    w1t = nc.dram_tensor("w1t", [KC, 128, D], BF16, kind="ExternalInput")
    w2t = nc.dram_tensor("w2t", [KC, 128, D], BF16, kind="ExternalInput")
    b1c = nc.dram_tensor("b1c", [128, KC], F32, kind="ExternalInput")
    b2c = nc.dram_tensor("b2c", [128, KC], F32, kind="ExternalInput")

    rs_out = nc.dram_tensor("rs", [128, 3 * MT * BB], F32, kind="ExternalOutput")
    acc_out = nc.dram_tensor("accsum", [128, N], F32, kind="ExternalOutput")
    d12_out = nc.dram_tensor("d12", [2, 512], F32, kind="ExternalOutput")

    # per-e gather buffers (separate tensors so the two collectives carry no
    # false dependencies on each other)
    n_all = [nc.dram_tensor(f"n_all{e}", [NCORES, 128, KC, R], F8,
                            addr_space="Shared") for e in range(2)]
    warm_out = nc.dram_tensor("warm_out", [NCORES, 64], F8, addr_space="Shared")

    EXP = mybir.ActivationFunctionType.Exp
    RELU = mybir.ActivationFunctionType.Relu
    LNF = mybir.ActivationFunctionType.Ln
    IDENT = mybir.ActivationFunctionType.Identity
    DR = mybir.MatmulPerfMode.DoubleRow

    with tile.TileContext(nc) as tc:
        with tc.tile_pool(name="keep", bufs=1) as kp, \
             tc.tile_pool(name="dr", bufs=1, space="DRAM") as dr:

            # ---- persistent tiles ----
            b1s = kp.tile([128, KC], F32, name="b1s", tag="b1s")
            b2s = kp.tile([128, KC], F32, name="b2s", tag="b2s")
            nc.sync.dma_start(out=b1s, in_=b1c[:, :])
            nc.sync.dma_start(out=b2s, in_=b2c[:, :])
            ones_k = kp.tile([128, 1], F32, name="ones_k", tag="ones_k")
            nc.vector.memset(ones_k, 1.0)
            rs = kp.tile([128, 3 * MT * BB], F32, name="rs", tag="rs")
            nc.vector.memset(rs, 0.0)
            # fp8 normalized tiles (x16), contraction chunk on dim1
            nt8 = [kp.tile([128, KC, R], F8, name=f"nt8_{e}", tag=f"nt8_{e}")
                   for e in range(2)]
            mp = kp.tile([128, R], F32, name="mp", tag="mp")
            n_loc = [dr.tile([128, KC, R], F8, name=f"n_loc{e}", tag=f"n_loc{e}")
                     for e in range(2)]

            # tiny warm-up gather: pays the one-time RDH ring setup (~11us)
            # during the projection so the real gathers trigger fast
            warm_in = dr.tile([1, 64], F8, name="warm_in", tag="warm_in")
            if STAGE >= 2:
                nc.gpsimd.collective_compute(
                    "AllGather", mybir.AluOpType.bypass,
                    replica_groups=[list(range(NCORES))],
                    ins=[warm_in[:].opt()],
                    outs=[warm_out[:].opt()])

            # ---- projection + normalize (scoped pool) ----
            with tc.tile_pool(name="proj", bufs=1) as pj, \
                 tc.tile_pool(name="psp", bufs=1, space="PSUM") as psp:
                # batched input DMAs, ordered so e0's operands land first
                w1b = pj.tile([128, KC, D], BF16, name="w1b", tag="w1b")
                w2b = pj.tile([128, KC, D], BF16, name="w2b", tag="w2b")
                ztb = [pj.tile([128, KC, R], BF16, name=f"ztb{e}",
                               tag=f"ztb{e}") for e in range(2)]
                nc.sync.dma_start(out=w1b, in_=w1t[:])
                nc.sync.dma_start(out=ztb[0], in_=z1t[:])
                nc.sync.dma_start(out=ztb[1], in_=z2t[:])
                nc.sync.dma_start(out=w2b, in_=w2t[:])
                w1 = [w1b[:, k, :] for k in range(KC)]
                w2 = [w2b[:, k, :] for k in range(KC)]
                zt = [[ztb[e][:, k, :] for k in range(KC)] for e in range(2)]
                # broadcast vector carries the fp8 16x scale: bc = 16/|h|
                ones_b = pj.tile([1, 128], F32, name="ones_b", tag="ones_b")
                nc.vector.memset(ones_b, FP8_SCALE)

                et = [[pj.tile([128, R], BF16, name=f"et{e}_{k}",
                               tag=f"et{e}_{k}") for k in range(KC)]
                      for e in range(2)]
                ht = [[pj.tile([128, R], F32, name=f"ht{e}_{k}",
                               tag=f"ht{e}_{k}") for k in range(KC)]
                      for e in range(2)]
                nsq = [pj.tile([128, R], F32, name=f"nsq{e}", tag=f"nsq{e}")
                       for e in range(2)]

                def layer1(e):
                    # layer 1 + elu (elu = min(exp(x)-1, relu(x)))
                    for oc in range(KC):
                        pa = psp.tile([128, R], F32, name="pa", tag="pa", bufs=2)
                        for h in range(R // 512):
                            for k in range(KC):
                                nc.tensor.matmul(
                                    pa[:, h * 512:(h + 1) * 512],
                                    w1[k][:, oc * 128:(oc + 1) * 128],
                                    zt[e][k][:, h * 512:(h + 1) * 512],
                                    start=(k == 0), stop=(k == KC - 1))
                        t1 = pj.tile([128, R], F32, name="t1", tag="t1", bufs=2)
                        t2 = pj.tile([128, R], F32, name="t2", tag="t2", bufs=2)
                        nc.scalar.activation(t1, pa, EXP, bias=b1s[:, oc:oc + 1])
                        nc.scalar.activation(t2, pa, RELU, bias=b1s[:, oc:oc + 1])
                        nc.vector.scalar_tensor_tensor(
                            et[e][oc], t1, 1.0, t2,
                            mybir.AluOpType.subtract, mybir.AluOpType.min)

                def layer2(e):
                    # layer 2 + bias; squared norms fused per block
                    for pc in range(KC):
                        ph = psp.tile([128, R], F32, name="pa", tag="pa", bufs=2)
                        for h in range(R // 512):
                            for k in range(KC):
                                nc.tensor.matmul(
                                    ph[:, h * 512:(h + 1) * 512],
                                    w2[k][:, pc * 128:(pc + 1) * 128],
                                    et[e][k][:, h * 512:(h + 1) * 512],
                                    start=(k == 0), stop=(k == KC - 1))
                        nc.scalar.activation(ht[e][pc], ph, IDENT,
                                             bias=b2s[:, pc:pc + 1])
                        if pc == 0:
                            nc.vector.tensor_mul(nsq[e], ht[e][0], ht[e][0])
                        else:
                            sq = pj.tile([128, R], F32, name="t1",
                                         tag="t1", bufs=2)
                            nc.vector.tensor_mul(sq, ht[e][pc], ht[e][pc])
                            nc.vector.tensor_add(nsq[e], nsq[e], sq)

                def normalize_and_gather(e):
                    # 16/norm via exp(-0.5*ln(sum h^2)); broadcast; fp8 cast
                    nrm = psp.tile([1, R], F32, name="nrm", tag="nrm", bufs=1)
                    for h in range(R // 512):
                        nc.tensor.matmul(nrm[0:1, h * 512:(h + 1) * 512],
                                         ones_k,
                                         nsq[e][:, h * 512:(h + 1) * 512],
                                         start=True, stop=True)
                    snrm = pj.tile([1, R], F32, name="snrm", tag="snrm", bufs=2)
                    nc.vector.tensor_copy(snrm, nrm)
                    sr = pj.tile([1, R], F32, name="sr", tag="sr", bufs=2)
                    nc.scalar.activation(sr, snrm, LNF)
                    nc.scalar.activation(sr, sr, EXP, scale=-0.5)
                    bc = psp.tile([128, R], F32, name="bc", tag="bc", bufs=1)
                    for h in range(R // 512):
                        nc.tensor.matmul(bc[:, h * 512:(h + 1) * 512],
                                         ones_b,
                                         sr[0:1, h * 512:(h + 1) * 512],
                                         start=True, stop=True)
                    for pc in range(KC):
                        nc.vector.tensor_mul(nt8[e][:, pc, :], ht[e][pc], bc)
                    # DMA from the gpsimd queue so the sync queue (res tile
                    # loads) never waits behind this trigger
                    nc.gpsimd.dma_start(out=n_loc[e][:], in_=nt8[e][:])
                    if STAGE >= 2:
                        nc.gpsimd.collective_compute(
                            "AllGather", mybir.AluOpType.bypass,
                            replica_groups=[list(range(NCORES))],
                            ins=[n_loc[e][:].opt()],
                            outs=[n_all[e][:].opt()])

                # e0 depth-first so gather0 triggers as early as possible,
                # then all of e1 (its gather has plenty of slack)
                layer1(0)
                layer2(0)
                normalize_and_gather(0)
                layer1(1)
                layer2(1)
                normalize_and_gather(1)

                # d12 row-dot products from the fp8 tiles (256x scale)
                m2 = pj.tile([128, R], F32, name="t2", tag="t2", bufs=2)
                nc.vector.tensor_mul(mp, nt8[0][:, 0, :], nt8[1][:, 0, :])
                for k in range(1, KC):
                    nc.vector.tensor_mul(m2, nt8[0][:, k, :], nt8[1][:, k, :])
                    nc.vector.tensor_add(mp, mp, m2)

            # ---- main similarity loops (scoped pool) ----
            with tc.tile_pool(name="main", bufs=1) as mn:
              with tc.tile_pool(name="psm", bufs=1, space="PSUM") as psm:
                acc = mn.tile([128, N], F32, name="acc", tag="acc")
                nc.vector.memset(acc, 0.0)

                # d12 partition-reduce through a pg-slot while the tensor
                # engine is otherwise waiting on gather0
                dpg = psm.tile([128, 2048], F32, name="pg", tag="pg", bufs=2)
                for h in range(2):
                    nc.tensor.matmul(dpg[0:1, h * 512:(h + 1) * 512], ones_k,
                                     mp[:, h * 512:(h + 1) * 512],
                                     start=True, stop=True)
                dstg = mn.tile([1, 1024], F32, name="dstg", tag="dstg")
                nc.vector.tensor_copy(dstg, dpg[0:1, 0:1024])
                nc.sync.dma_start(out=d12_out[:, :], in_=dstg)

                def mm_group(pg, own, res, m):
                    for t in range(4):
                        j, half = t // 2, t % 2
                        for kq in range(2):
                            nc.tensor.matmul(
                                pg[:, t * 512:(t + 1) * 512],
                                own[:, 2 * kq:2 * kq + 2, m * 128:(m + 1) * 128],
                                res[j][:, 2 * kq:2 * kq + 2,
                                       half * 512:(half + 1) * 512],
                                start=(kq == 0), stop=(kq == 1),
                                perf_mode=DR)

                ESC = 2.0 / (FP8_SCALE * FP8_SCALE)  # psum holds 256*S

                # phase B: S11 (rhs = gathered n1)
                for bb in (range(BB) if STAGE >= 2 else []):
                    res = [mn.tile([128, KC, R], F8, name=f"r{j}",
                                   tag=f"r{j}", bufs=2) for j in range(2)]
                    for j in range(2):
                        nc.sync.dma_start(out=res[j][:],
                                          in_=n_all[0][2 * bb + j])
                    for m in range(MT):
                        pg = psm.tile([128, 2048], F32, name="pg", tag="pg",
                                      bufs=2)
                        mm_group(pg, nt8[0], res, m)
                        col = 0 * (MT * BB) + m * BB + bb
                        nc.scalar.activation(pg, pg, EXP, scale=ESC,
                                             accum_out=rs[:, col:col + 1])

                # phase CD: S12 + S22 (rhs = gathered n2)
                for bb in (range(BB) if STAGE >= 3 else []):
                    res = [mn.tile([128, KC, R], F8, name=f"r{j}",
                                   tag=f"r{j}", bufs=2) for j in range(2)]
                    for j in range(2):
                        nc.sync.dma_start(out=res[j][:],
                                          in_=n_all[1][2 * bb + j])
                    for m in range(MT):
                        pg = psm.tile([128, 2048], F32, name="pg", tag="pg",
                                      bufs=2)
                        mm_group(pg, nt8[0], res, m)
                        col = 1 * (MT * BB) + m * BB + bb
                        # stage exp in SBUF so the PSUM buffer frees at
                        # ACT-time and the column-sum add runs off-path
                        ex = mn.tile([128, 2048], F32, name="ex", tag="ex",
                                     bufs=2)
                        nc.scalar.activation(ex, pg, EXP, scale=ESC,
                                             accum_out=rs[:, col:col + 1])
                        nc.vector.tensor_add(acc[:, bb * 2048:(bb + 1) * 2048],
                                             acc[:, bb * 2048:(bb + 1) * 2048],
                                             ex)

                        pg2 = psm.tile([128, 2048], F32, name="pg", tag="pg",
                                       bufs=2)
                        mm_group(pg2, nt8[1], res, m)
                        col = 2 * (MT * BB) + m * BB + bb
                        nc.scalar.activation(pg2, pg2, EXP, scale=ESC,
                                             accum_out=rs[:, col:col + 1])

                    # this bb's column block of acc is complete: ship it to
                    # DRAM on idle DMA bandwidth; the 128-way partition
                    # reduction happens in the host assembly
                    nc.sync.dma_start(out=acc_out[:, bb * 2048:(bb + 1) * 2048],
                                      in_=acc[:, bb * 2048:(bb + 1) * 2048])
            nc.sync.dma_start(out=rs_out[:, :], in_=rs)

    nc.compile()
    return nc


def _get_nc():
    if "nc" not in _CACHE:
        _CACHE["nc"] = _build()
    return _CACHE["nc"]


def make_in_maps(pri, aux, W1, b1, W2, b2):
    pri = np.asarray(pri, dtype=np.float32)
    aux = np.asarray(aux, dtype=np.float32)
    w1t = np.ascontiguousarray(
        np.asarray(W1, dtype=np.float32).T).astype(
            ml_dtypes.bfloat16).reshape(KC, 128, D)
    w2t = np.ascontiguousarray(
        np.asarray(W2, dtype=np.float32).T).astype(
            ml_dtypes.bfloat16).reshape(KC, 128, D)
    b1 = np.asarray(b1, dtype=np.float32)
    b2 = np.asarray(b2, dtype=np.float32)
    b1c = np.ascontiguousarray(b1.reshape(KC, 128).T)
    b2c = np.ascontiguousarray(b2.reshape(KC, 128).T)
    priT = np.ascontiguousarray(pri.T).astype(ml_dtypes.bfloat16)
    auxT = np.ascontiguousarray(aux.T).astype(ml_dtypes.bfloat16)

    in_maps = []
    for c in range(NCORES):
        sl = slice(c * R, (c + 1) * R)
        in_maps.append({
            "z1t": np.ascontiguousarray(priT[:, sl]).reshape(KC, 128, R),
            "z2t": np.ascontiguousarray(auxT[:, sl]).reshape(KC, 128, R),
            "w1t": w1t, "w2t": w2t, "b1c": b1c, "b2c": b2c,
        })
    return in_maps


def assemble(results):
    """CPU assembly of the scalar loss from per-core partials"""
    E2 = np.exp(np.float64(2.0))
    SC2 = np.float64(FP8_SCALE * FP8_SCALE)
    colsum_full = np.zeros(N, dtype=np.float64)
    for c in range(NCORES):
        colsum_full += results[c]["accsum"].astype(np.float64).sum(axis=0)

    total = np.float64(0.0)
    for c in range(NCORES):
        rs = results[c]["rs"].astype(np.float64)      # [128, 96]
        r = rs.reshape(128, 3, MT, BB).sum(-1)        # [128, 3, MT]
        # row i_local = m*128 + p  ->  transpose to [MT, 128] then flatten
        rs11 = r[:, 0, :].T.reshape(R)
        rs12 = r[:, 1, :].T.reshape(R)
        rs22 = r[:, 2, :].T.reshape(R)
        d12 = results[c]["d12"].astype(np.float64).reshape(R) / SC2
        den1 = rs11 + rs12 - E2
        den2 = rs22 + colsum_full[c * R:(c + 1) * R] - E2
        li = 0.5 * (np.log(den1) + np.log(den2)) - 2.0 * d12
        total += li.sum()

    return np.float32(total / N)


def kernel(pri_embedding, aux_embedding, W1, b1, W2, b2):
    in_maps = make_in_maps(pri_embedding, aux_embedding, W1, b1, W2, b2)
    nc = _get_nc()
    res = run_bass_kernel_spmd(nc, in_maps, list(range(NCORES))).results
    return assemble(res)


# revision 23
# speedup vs baseline: 1.1122x; 1.0183x over previous
"""Distributed Trainium2 kernel for nn_Contrast_loss (row-parallel InfoNCE).

Math (reference):
  h1 = proj(pri), h2 = proj(aux)   with proj(z) = elu(z@W1.T+b1)@W2.T+b2
  n1 = normalize(h1), n2 = normalize(h2)
  l1_i = log(den1_i) - 2*d12_i,  den1_i = sum_j e^{2 S11_ij} + sum_j e^{2 S12_ij} - e^{2 S11_ii}
  l2_i = log(den2_i) - 2*d12_i,  den2_i = sum_j e^{2 S22_ij} + sum_j e^{2 S12_ji} - e^{2 S22_ii}
  loss = mean((l1+l2)/2)
  (S11_ii = S22_ii = 1 since rows are unit-normalized; d12_i = n1_i . n2_i)

Sharding: rows split across 8 cores (1024 rows each). Each core projects +
normalizes its row block (transposed layout [D, rows]) in bf16 matmuls, then
stores 16*n as fp8e4 tiles laid out [128, KC, R] (contraction chunk in dim1).
The two normalized matrices are AllGathered separately (4MB each) so the
second gather overlaps the S11 phase. Similarity row-blocks use fp8 DoubleRow
matmuls (2 contraction chunks per instruction, 2x PE throughput); exp(2x) and
row sums are fused on the scalar engine (activation accum_out, scale=2/256
since fp8 values carry a 16x scale -> psum holds 256*S). S12 column partials
accumulate on the vector engine and are partition-reduced with ones-matmuls.
Per-core partial sums are assembled into the scalar loss on CPU (O(N) work).

Normalization uses 1/sqrt(x) = exp(-0.5*ln(x) + ln(16)) so every activation
in the kernel (elu's exp/relu, ln, exp, identity) lives in the single
natural_log_exp_and_others table set -- no table reloads.
"""

import os
import numpy as np
import ml_dtypes

import concourse.bass as bass
import concourse.tile as tile
from concourse import mybir, bacc, bass_isa
from concourse.bass_utils import run_bass_kernel_spmd

NCORES = 8
N = 8192
D = 512
R = N // NCORES          # rows per core = 1024
KC = D // 128            # contraction chunks = 4
MT = R // 128            # row tiles per core = 8
BB = 4                   # column super-blocks (each = 2048 cols = 2 source cores)
F32 = mybir.dt.float32
BF16 = mybir.dt.bfloat16
F8 = mybir.dt.float8e4

FP8_SCALE = 16.0         # normalized values stored as 16*n in fp8e4
LN_SCALE = float(np.log(FP8_SCALE))

_CACHE = {}


def _build():
    STAGE = int(os.environ.get("BASS_STAGE", "4"))
    nc = bacc.Bacc("TRN2", target_bir_lowering=False, debug=False,
                   num_devices=NCORES)

    z1t = nc.dram_tensor("z1t", [KC, 128, R], BF16, kind="ExternalInput")
    z2t = nc.dram_tensor("z2t", [KC, 128, R], BF16, kind="ExternalInput")
    w1t = nc.dram_tensor("w1t", [KC, 128, D], BF16, kind="ExternalInput")
    w2t = nc.dram_tensor("w2t", [KC, 128, D], BF16, kind="ExternalInput")
    b1c = nc.dram_tensor("b1c", [128, KC], F32, kind="ExternalInput")
    b2c = nc.dram_tensor("b2c", [128, KC], F32, kind="ExternalInput")

    rs_out = nc.dram_tensor("rs", [128, 3 * MT * BB], F32, kind="ExternalOutput")
    acc_out = nc.dram_tensor("accsum", [128, N], F32, kind="ExternalOutput")
    d12_out = nc.dram_tensor("d12", [2, 512], F32, kind="ExternalOutput")

    # per-e gather buffers (separate tensors so the two collectives carry no
    # false dependencies on each other)
    n_all = [nc.dram_tensor(f"n_all{e}", [NCORES, 128, KC, R], F8,
                            addr_space="Shared") for e in range(2)]
    warm_out = nc.dram_tensor("warm_out", [NCORES, 64], F8, addr_space="Shared")

    EXP = mybir.ActivationFunctionType.Exp
    RELU = mybir.ActivationFunctionType.Relu
    LNF = mybir.ActivationFunctionType.Ln
    IDENT = mybir.ActivationFunctionType.Identity
    DR = mybir.MatmulPerfMode.DoubleRow

    with tile.TileContext(nc) as tc:
        with tc.tile_pool(name="keep", bufs=1) as kp, \
             tc.tile_pool(name="dr", bufs=1, space="DRAM") as dr:

            # ---- persistent tiles ----
            b1s = kp.tile([128, KC], F32, name="b1s", tag="b1s")
            b2s = kp.tile([128, KC], F32, name="b2s", tag="b2s")
            nc.sync.dma_start(out=b1s, in_=b1c[:, :])
            nc.sync.dma_start(out=b2s, in_=b2c[:, :])
            ones_k = kp.tile([128, 1], F32, name="ones_k", tag="ones_k")
            nc.vector.memset(ones_k, 1.0)
            rs = kp.tile([128, 3 * MT * BB], F32, name="rs", tag="rs")
            nc.vector.memset(rs, 0.0)
            # fp8 normalized tiles (x16), contraction chunk on dim1
            nt8 = [kp.tile([128, KC, R], F8, name=f"nt8_{e}", tag=f"nt8_{e}")
                   for e in range(2)]
            mp = kp.tile([128, R], F32, name="mp", tag="mp")
            n_loc = [dr.tile([128, KC, R], F8, name=f"n_loc{e}", tag=f"n_loc{e}")
                     for e in range(2)]

            # tiny warm-up gather: pays the one-time RDH ring setup (~11us)
            # during the projection so the real gathers trigger fast
            warm_in = dr.tile([1, 64], F8, name="warm_in", tag="warm_in")
            if STAGE >= 2:
                nc.gpsimd.collective_compute(
                    "AllGather", mybir.AluOpType.bypass,
                    replica_groups=[list(range(NCORES))],
                    ins=[warm_in[:].opt()],
                    outs=[warm_out[:].opt()])

            # ---- projection + normalize (scoped pool) ----
            with tc.tile_pool(name="proj", bufs=1) as pj, \
                 tc.tile_pool(name="psp", bufs=1, space="PSUM") as psp:
                # batched input DMAs, ordered so e0's operands land first
                w1b = pj.tile([128, KC, D], BF16, name="w1b", tag="w1b")
                w2b = pj.tile([128, KC, D], BF16, name="w2b", tag="w2b")
                ztb = [pj.tile([128, KC, R], BF16, name=f"ztb{e}",
                               tag=f"ztb{e}") for e in range(2)]
                nc.sync.dma_start(out=w1b, in_=w1t[:])
                nc.sync.dma_start(out=ztb[0], in_=z1t[:])
                nc.sync.dma_start(out=ztb[1], in_=z2t[:])
                nc.sync.dma_start(out=w2b, in_=w2t[:])
                w1 = [w1b[:, k, :] for k in range(KC)]
                w2 = [w2b[:, k, :] for k in range(KC)]
                zt = [[ztb[e][:, k, :] for k in range(KC)] for e in range(2)]
                # broadcast vector carries the fp8 16x scale: bc = 16/|h|
                ones_b = pj.tile([1, 128], F32, name="ones_b", tag="ones_b")
                nc.vector.memset(ones_b, FP8_SCALE)

                et = [[pj.tile([128, R], BF16, name=f"et{e}_{k}",
                               tag=f"et{e}_{k}") for k in range(KC)]
                      for e in range(2)]
                ht = [[pj.tile([128, R], F32, name=f"ht{e}_{k}",
                               tag=f"ht{e}_{k}") for k in range(KC)]
                      for e in range(2)]
                nsq = [pj.tile([128, R], F32, name=f"nsq{e}", tag=f"nsq{e}")
                       for e in range(2)]

                def layer1(e):
                    # layer 1 + elu (elu = min(exp(x)-1, relu(x)))
                    for oc in range(KC):
                        pa = psp.tile([128, R], F32, name="pa", tag="pa", bufs=2)
                        for h in range(R // 512):
                            for k in range(KC):
                                nc.tensor.matmul(
                                    pa[:, h * 512:(h + 1) * 512],
                                    w1[k][:, oc * 128:(oc + 1) * 128],
                                    zt[e][k][:, h * 512:(h + 1) * 512],
                                    start=(k == 0), stop=(k == KC - 1))
                        t1 = pj.tile([128, R], F32, name="t1", tag="t1", bufs=2)
                        t2 = pj.tile([128, R], F32, name="t2", tag="t2", bufs=2)
                        nc.scalar.activation(t1, pa, EXP, bias=b1s[:, oc:oc + 1])
                        nc.scalar.activation(t2, pa, RELU, bias=b1s[:, oc:oc + 1])
                        nc.vector.scalar_tensor_tensor(
                            et[e][oc], t1, 1.0, t2,
                            mybir.AluOpType.subtract, mybir.AluOpType.min)

                def layer2(e):
                    # layer 2 + bias; squared norms fused per block
                    for pc in range(KC):
                        ph = psp.tile([128, R], F32, name="pa", tag="pa", bufs=2)
                        for h in range(R // 512):
                            for k in range(KC):
                                nc.tensor.matmul(
                                    ph[:, h * 512:(h + 1) * 512],
                                    w2[k][:, pc * 128:(pc + 1) * 128],
                                    et[e][k][:, h * 512:(h + 1) * 512],
                                    start=(k == 0), stop=(k == KC - 1))
                        nc.scalar.activation(ht[e][pc], ph, IDENT,
                                             bias=b2s[:, pc:pc + 1])
                        if pc == 0:
                            nc.vector.tensor_mul(nsq[e], ht[e][0], ht[e][0])
                        else:
                            sq = pj.tile([128, R], F32, name="t1",
                                         tag="t1", bufs=2)
                            nc.vector.tensor_mul(sq, ht[e][pc], ht[e][pc])
                            nc.vector.tensor_add(nsq[e], nsq[e], sq)

                def normalize_and_gather(e):
                    # 16/norm via exp(-0.5*ln(sum h^2)); broadcast; fp8 cast
                    nrm = psp.tile([1, R], F32, name="nrm", tag="nrm", bufs=1)
                    for h in range(R // 512):
                        nc.tensor.matmul(nrm[0:1, h * 512:(h + 1) * 512],
                                         ones_k,
                                         nsq[e][:, h * 512:(h + 1) * 512],
                                         start=True, stop=True)
                    sr = pj.tile([1, R], F32, name="sr", tag="sr", bufs=2)
                    nc.scalar.activation(sr, nrm, LNF)
                    nc.scalar.activation(sr, sr, EXP, scale=-0.5)
                    bc = psp.tile([128, R], F32, name="bc", tag="bc", bufs=1)
                    for h in range(R // 512):
                        nc.tensor.matmul(bc[:, h * 512:(h + 1) * 512],
                                         ones_b,
                                         sr[0:1, h * 512:(h + 1) * 512],
                                         start=True, stop=True)
                    for pc in range(KC):
                        nc.vector.tensor_mul(nt8[e][:, pc, :], ht[e][pc], bc)
                    # DMA from the gpsimd queue so the sync queue (res tile
                    # loads) never waits behind this trigger
                    nc.gpsimd.dma_start(out=n_loc[e][:], in_=nt8[e][:])
                    if STAGE >= 2:
                        nc.gpsimd.collective_compute(
                            "AllGather", mybir.AluOpType.bypass,
                            replica_groups=[list(range(NCORES))],
                            ins=[n_loc[e][:].opt()],
                            outs=[n_all[e][:].opt()])

                # e0 depth-first so gather0 triggers as early as possible,
                # then all of e1 (its gather has plenty of slack)
                layer1(0)
                layer2(0)
                normalize_and_gather(0)
                layer1(1)
                layer2(1)
                normalize_and_gather(1)

                # d12 row-dot products from the fp8 tiles (256x scale)
                m2 = pj.tile([128, R], F32, name="t2", tag="t2", bufs=2)
                nc.vector.tensor_mul(mp, nt8[0][:, 0, :], nt8[1][:, 0, :])
                for k in range(1, KC):
                    nc.vector.tensor_mul(m2, nt8[0][:, k, :], nt8[1][:, k, :])
                    nc.vector.tensor_add(mp, mp, m2)

            # ---- main similarity loops (scoped pool) ----
            with tc.tile_pool(name="main", bufs=1) as mn:
              with tc.tile_pool(name="psm", bufs=1, space="PSUM") as psm:
                acc = mn.tile([128, N], F32, name="acc", tag="acc")
                nc.vector.memset(acc, 0.0)

                # d12 partition-reduce through a pg-slot while the tensor
                # engine is otherwise waiting on gather0
                dpg = psm.tile([128, 2048], F32, name="pg", tag="pg", bufs=2)
                for h in range(2):
                    nc.tensor.matmul(dpg[0:1, h * 512:(h + 1) * 512], ones_k,
                                     mp[:, h * 512:(h + 1) * 512],
                                     start=True, stop=True)
                dstg = mn.tile([1, 1024], F32, name="dstg", tag="dstg")
                nc.vector.tensor_copy(dstg, dpg[0:1, 0:1024])
                nc.sync.dma_start(out=d12_out[:, :], in_=dstg)

                def mm_group(pg, own, res, m):
                    for t in range(4):
                        j, half = t // 2, t % 2
                        for kq in range(2):
                            nc.tensor.matmul(
                                pg[:, t * 512:(t + 1) * 512],
                                own[:, 2 * kq:2 * kq + 2, m * 128:(m + 1) * 128],
                                res[j][:, 2 * kq:2 * kq + 2,
                                       half * 512:(half + 1) * 512],
                                start=(kq == 0), stop=(kq == 1),
                                perf_mode=DR)

                ESC = 2.0 / (FP8_SCALE * FP8_SCALE)  # psum holds 256*S

                # phase B: S11 (rhs = gathered n1)
                for bb in (range(BB) if STAGE >= 2 else []):
                    res = [mn.tile([128, KC, R], F8, name=f"r{j}",
                                   tag=f"r{j}", bufs=2) for j in range(2)]
                    for j in range(2):
                        nc.sync.dma_start(out=res[j][:],
                                          in_=n_all[0][2 * bb + j])
                    for m in range(MT):
                        pg = psm.tile([128, 2048], F32, name="pg", tag="pg",
                                      bufs=2)
                        mm_group(pg, nt8[0], res, m)
                        col = 0 * (MT * BB) + m * BB + bb
                        nc.scalar.activation(pg, pg, EXP, scale=ESC,
                                             accum_out=rs[:, col:col + 1])

                # phase CD: S12 + S22 (rhs = gathered n2)
                for bb in (range(BB) if STAGE >= 3 else []):
                    res = [mn.tile([128, KC, R], F8, name=f"r{j}",
                                   tag=f"r{j}", bufs=2) for j in range(2)]
                    for j in range(2):
                        nc.sync.dma_start(out=res[j][:],
                                          in_=n_all[1][2 * bb + j])
                    for m in range(MT):
                        pg = psm.tile([128, 2048], F32, name="pg", tag="pg",
                                      bufs=2)
                        mm_group(pg, nt8[0], res, m)
                        col = 1 * (MT * BB) + m * BB + bb
                        # stage exp in SBUF so the PSUM buffer frees at
                        # ACT-time and the column-sum add runs off-path
                        ex = mn.tile([128, 2048], F32, name="ex", tag="ex",
                                     bufs=2)
                        nc.scalar.activation(ex, pg, EXP, scale=ESC,
                                             accum_out=rs[:, col:col + 1])
                        nc.vector.tensor_add(acc[:, bb * 2048:(bb + 1) * 2048],
                                             acc[:, bb * 2048:(bb + 1) * 2048],
                                             ex)

                        pg2 = psm.tile([128, 2048], F32, name="pg", tag="pg",
                                       bufs=2)
                        mm_group(pg2, nt8[1], res, m)
                        col = 2 * (MT * BB) + m * BB + bb
                        nc.scalar.activation(pg2, pg2, EXP, scale=ESC,
                                             accum_out=rs[:, col:col + 1])

                    # this bb's column block of acc is complete: ship it to
                    # DRAM on idle DMA bandwidth; the 128-way partition
                    # reduction happens in the host assembly
                    nc.sync.dma_start(out=acc_out[:, bb * 2048:(bb + 1) * 2048],
                                      in_=acc[:, bb * 2048:(bb + 1) * 2048])
            nc.sync.dma_start(out=rs_out[:, :], in_=rs)

    nc.compile()
    return nc


def _get_nc():
    if "nc" not in _CACHE:
        _CACHE["nc"] = _build()
    return _CACHE["nc"]


def make_in_maps(pri, aux, W1, b1, W2, b2):
    pri = np.asarray(pri, dtype=np.float32)
    aux = np.asarray(aux, dtype=np.float32)
    w1t = np.ascontiguousarray(
        np.asarray(W1, dtype=np.float32).T).astype(
            ml_dtypes.bfloat16).reshape(KC, 128, D)
    w2t = np.ascontiguousarray(
        np.asarray(W2, dtype=np.float32).T).astype(
            ml_dtypes.bfloat16).reshape(KC, 128, D)
    b1 = np.asarray(b1, dtype=np.float32)
    b2 = np.asarray(b2, dtype=np.float32)
    b1c = np.ascontiguousarray(b1.reshape(KC, 128).T)
    b2c = np.ascontiguousarray(b2.reshape(KC, 128).T)
    priT = np.ascontiguousarray(pri.T).astype(ml_dtypes.bfloat16)
    auxT = np.ascontiguousarray(aux.T).astype(ml_dtypes.bfloat16)

    in_maps = []
    for c in range(NCORES):
        sl = slice(c * R, (c + 1) * R)
        in_maps.append({
            "z1t": np.ascontiguousarray(priT[:, sl]).reshape(KC, 128, R),
            "z2t": np.ascontiguousarray(auxT[:, sl]).reshape(KC, 128, R),
            "w1t": w1t, "w2t": w2t, "b1c": b1c, "b2c": b2c,
        })
    return in_maps


def assemble(results):
    """CPU assembly of the scalar loss from per-core partials"""
    E2 = np.exp(np.float64(2.0))
    SC2 = np.float64(FP8_SCALE * FP8_SCALE)
    colsum_full = np.zeros(N, dtype=np.float64)
    for c in range(NCORES):
        colsum_full += results[c]["accsum"].astype(np.float64).sum(axis=0)

    total = np.float64(0.0)
    for c in range(NCORES):
        rs = results[c]["rs"].astype(np.float64)      # [128, 96]
        r = rs.reshape(128, 3, MT, BB).sum(-1)        # [128, 3, MT]
        # row i_local = m*128 + p  ->  transpose to [MT, 128] then flatten
        rs11 = r[:, 0, :].T.reshape(R)
        rs12 = r[:, 1, :].T.reshape(R)
        rs22 = r[:, 2, :].T.reshape(R)
        d12 = results[c]["d12"].astype(np.float64).reshape(R) / SC2
        den1 = rs11 + rs12 - E2
        den2 = rs22 + colsum_full[c * R:(c + 1) * R] - E2
        li = 0.5 * (np.log(den1) + np.log(den2)) - 2.0 * d12
        total += li.sum()

    return np.float32(total / N)


def kernel(pri_embedding, aux_embedding, W1, b1, W2, b2):
    in_maps = make_in_maps(pri_embedding, aux_embedding, W1, b1, W2, b2)
    nc = _get_nc()
    res = run_bass_kernel_spmd(nc, in_maps, list(range(NCORES))).results
    return assemble(res)
